# revision 46
# baseline (speedup 1.0000x reference)
"""Trainium2 Bass kernel for CrossModalAttention (v3).

Reference computation (B=1, C=64, N=8192 voxels):
  two cross-attention directions (CT queries over MRI keys/values and vice
  versa), each with an 8192x8192 attention matrix, fused output projection.

Sharding: each of the 8 cores owns 1024 query voxels for BOTH directions,
computes K/V over the full sequence locally (features are only 2 MB per
modality), and produces its own (64, 1024) slice of the output. No
collectives; the host concatenates the 8 slices.

v3 key idea -- row-tiled score pairs. On this part the PE effectively runs
at 1.2 GHz (measured: pure back-to-back matmuls never leave K=4/8), so the
score matmuls (output-drain-bound) dominate. The K-projection bias adds a
per-QUERY constant to every score, which softmax cancels, so the score
contraction is exactly k=64 -- half the PE array. Both directions' score
matmuls for the same key chunk are issued as a concurrent row-tiled pair:
direction 0 in array rows 0-63 (features+queries at partitions 0-63),
direction 1 in rows 64-127 (features+queries DMA'd to partitions 64-127).
The pair drains in ~520 cycles instead of 2x520 -- the dominant PE term
halves. One ScalarE ACTIVATE then exponentiates both directions' scores
(1024 cols) in one go.

exp is split 112/16 between ScalarE (exp(s'+ln16)) and VectorE, which
computes (t^2+1)^4 = 16*exp(s')*(1+O(s'^3/96)) with t = 1+s'/4 formed by
the chain's first tensor_scalar (mult 0.25, add 1) straight off PSUM; the
common 16x factor divides out in the softmax. The VectorE chain is
emitted one op per pair so the DVE FIFO never blocks a PSUM slot for
long, and its AV matmuls run at the end of the block (PSUM accumulation
is order-free after the start=True clear). Wo is folded into the V
projections on the host; V projections interleave into the pair loop as
deferred units.

Everything else follows the baseline "transposed world": K folded into
the query side, exp'd chunks become AV matmul weights with a ones-column
accumulating the softmax denominator, per-query normalize, PE transpose
back to channel-major. fp16 matmul operands, fp32 PSUM accumulation.
"""

from contextlib import ExitStack

import numpy as np

import concourse.bass as bass
import concourse.mybir as mybir
import concourse.tile as tile
from concourse import bacc
from concourse.bass_utils import run_bass_kernel_spmd

F32 = mybir.dt.float32
F16 = mybir.dt.float16
C = 64          # channels
N = 8192        # voxels (8*32*32)
NCORES = 8
NQ = N // NCORES      # 1024 queries per core
IH = 512              # query block (PSUM bank width in f32)
NIH = NQ // IH        # 2
NSUB = IH // 128      # 4 query subblocks per block (AV lhsT width)
JCH = 128             # key chunk (AV contraction tile)
NJ = N // JCH         # 64
LAG = 1               # pairs the AV matmuls trail the ScalarE exp by
VGW = 4               # vT chunks per projection group (4*65 f32, 1 bank)
W = C + 1             # 65: augmented channel dim
VCOLS = NJ * W        # vT storage: 64 chunks x 65 cols (65th col = denom)
NFS = 8               # feature DMA subtiles
FSW = N // NFS        # 1024 cols per subtile
JPS = FSW // JCH      # 8 j-chunks per feature subtile
LN16 = float(np.log(16.0))

# Per ih block: 64 chunk-pairs; 8 go through the VectorE/GpSimd squaring
# path, the rest through ScalarE.
PAIR_KIND = ["D" if (j % 8) == 6 else "A" for j in range(NJ)]
DLAG = 6  # pairs a D chunk's AV trails its (spread-out) exp chain by


def _emit_feat_load(nc, featp, mri_dram, ct_dram):
    """Interleave the two modalities' subtile DMAs so pair 0's d1 matmul
    (ct_hi subtile 0) isn't queued behind all 8 MRI subtiles."""
    fs_mri, fs_hi = [], []
    for s in range(NFS):
        t = featp.tile([W, FSW], F16, tag="fsm", name=f"fmri{s}")
        nc.sync.dma_start(t[:], mri_dram[:, FSW * s : FSW * (s + 1)])
        fs_mri.append(t)
        th = featp.tile([JCH, FSW], F16, tag="fsh", name=f"fcth{s}")
        nc.sync.dma_start(
            th[C : 2 * C, :], ct_dram[:C, FSW * s : FSW * (s + 1)]
        )
        fs_hi.append(th)
    return fs_mri, fs_hi


def _emit_qq_proj(nc, pools, wqq_sb, qsrc, wcol, name, hi_dst=None):
    """qq (64, NQ) = (Wq_aug @ Wk)^T @ qsrc_aug -- Q and (query-folded)
    K projections composed on the host; the score bias row is dropped
    (per-query constant, cancels in softmax). If hi_dst is given, the
    result is DMA-shifted to partitions 64-127 for the row-tiled pair."""
    qp, aux = pools["qp"], pools["aux"]
    qq = qp.tile([C, NQ], F16, tag="qq", name=name)

    def unit(h):
        def go():
            pq = aux.tile([C, IH], F32, tag="aux", name=f"pqq_{name}{h}")
            nc.tensor.matmul(
                pq[:],
                lhsT=wqq_sb[:, wcol : wcol + C],
                rhs=qsrc[:, IH * h : IH * (h + 1)],
                start=True,
                stop=True,
            )
            nc.vector.tensor_copy(qq[:, IH * h : IH * (h + 1)], pq[:])
            if hi_dst is not None:
                nc.sync.dma_start(
                    hi_dst[C : 2 * C, IH * h : IH * (h + 1)],
                    qq[:, IH * h : IH * (h + 1)],
                )
        return go

    return qq, [unit(h) for h in range(NIH)]


def _emit_v_proj(nc, pools, wv_sb, wv_hi, flo, fhi, vones, vpat):
    """vT (128j x 65) chunks = feat^T @ Wv' (Wo-folded, k=64: the V bias is
    a constant output shift -- softmax weights sum to 1 -- folded into bo on
    the host). Both modalities' projections are emitted as concurrent
    row-tiled pairs: MRI in array rows 0-63, CT in rows 64-127. The
    denominator ones-column of each chunk comes from a k=1 pattern matmul:
    the V matmuls leave those PSUM cells unwritten (first MM's start=True
    clears the whole bank), so the pattern matmul's start=False write
    lands 1.0 there and accumulates +0.0 onto the V columns."""
    vp, aux = pools["vp"], pools["aux"]
    vTs = [
        vp.tile([JCH, VCOLS], F16, tag="v", name=name)
        for name in ("vT_mri", "vT_ct")
    ]

    def unit(g):
        def go():
            pvs = []
            for d, (fs, wv, lo) in enumerate(
                [(flo, wv_sb, True), (fhi, wv_hi, False)]
            ):
                pv = aux.tile([JCH, VGW * W], F32, tag="aux", name=f"pv{d}_{g}")
                for cc in range(VGW):
                    j = VGW * g + cc
                    sl = slice(JCH * (j % JPS), JCH * (j % JPS + 1))
                    nc.tensor.matmul(
                        pv[:, W * cc : W * cc + C],
                        lhsT=fs[j // JPS][:C, sl] if lo
                        else fs[j // JPS][C : 2 * C, sl],
                        rhs=wv[:C, :C] if lo else wv[C : 2 * C, :C],
                        start=(cc == 0),
                        stop=False,
                        skip_group_check=True,
                    )
                nc.tensor.matmul(
                    pv[:],
                    lhsT=vones[:],
                    rhs=vpat[:],
                    start=False,
                    stop=True,
                    skip_group_check=True,
                )
                pvs.append(pv)
            for d in range(2):
                nc.vector.tensor_copy(
                    vTs[d][:, W * VGW * g : W * VGW * (g + 1)], pvs[d][:]
                )
        return go

    return vTs, [unit(g) for g in range(NJ // VGW)]


def _emit_block(nc, pools, feats, qqs, vTs, ih, deferred, finish):
    """One ih block: row-tiled score pairs -> exp (ACT or DVE) -> AV."""
    sp, pap, ep, epd, dp = (
        pools["sp"], pools["pap"], pools["ep"], pools["epd"], pools["dp"],
    )
    flo, fhi = feats
    qq_lo, qq_hi = qqs

    def emit_av(paccs, j, et):
        for d in range(2):
            for isub in range(NSUB):
                nc.tensor.matmul(
                    paccs[d][:, W * isub : W * (isub + 1)],
                    lhsT=et[:, IH * d + JCH * isub : IH * d + JCH * (isub + 1)],
                    rhs=vTs[d][:, W * j : W * (j + 1)],
                    start=(j == 0 and isub == 0),
                    stop=(j == NJ - 1 and isub == NSUB - 1),
                    skip_group_check=True,
                )

    paccs = [
        pap.tile([JCH, NSUB * W], F32, tag=f"pacc{d}", name=f"pacc{d}_{ih}")
        for d in range(2)
    ]
    pending = []
    dlate = []
    dve_work = []  # DVE chain ops, drained one per pair
    for j in range(NJ):
        kind = PAIR_KIND[j]
        ps = sp.tile([JCH, 2 * IH], F32, tag="ps", name=f"ps{ih}_{j}")
        sl = slice(JCH * (j % JPS), JCH * (j % JPS + 1))
        nc.tensor.matmul(
            ps[:, :IH],
            lhsT=flo[j // JPS][:C, sl],
            rhs=qq_lo[:, IH * ih : IH * (ih + 1)],
            start=True,
            stop=True,
        )
        nc.tensor.matmul(
            ps[:, IH:],
            lhsT=fhi[j // JPS][C : 2 * C, sl],
            rhs=qq_hi[C : 2 * C, IH * ih : IH * (ih + 1)],
            start=True,
            stop=True,
        )
        if kind == "A":
            et = ep.tile([JCH, 2 * IH], F16, tag="exp", name=f"et{ih}_{j}")
            # 16*exp(s'): the +ln16 matches the VectorE chunks' scale
            nc.scalar.activation(
                et[:],
                ps[:],
                mybir.ActivationFunctionType.Exp,
                bias=pools["ln16_sb"][:],
            )
            pending.append((paccs, j, et))
            if len(pending) > LAG:
                emit_av(*pending.pop(0))
        else:
            # t = 1 + s'/4 off PSUM in one tensor_scalar, then
            # (t^2+1)^4 = 16*exp(s')*(1+O(s'^3/96)) in fp16. The chain is
            # spread one op per pair so the DVE FIFO stays smooth; the
            # first op runs NOW to free the PSUM slot quickly.
            at = dp.tile([JCH, 2 * IH], F16, tag="da", name=f"a{ih}_{j}")
            nc.vector.tensor_scalar(
                at[:], ps[:], 0.25, 1.0,
                op0=mybir.AluOpType.mult, op1=mybir.AluOpType.add,
            )
            wt = dp.tile([JCH, 2 * IH], F16, tag="dw", name=f"w{ih}_{j}")
            xt = dp.tile([JCH, 2 * IH], F16, tag="dx", name=f"x{ih}_{j}")
            yt = dp.tile([JCH, 2 * IH], F16, tag="dy", name=f"y{ih}_{j}")
            et = epd.tile([JCH, 2 * IH], F16, tag="expd", name=f"etd{ih}_{j}")
            dve_work.extend([
                lambda a=at, w=wt: nc.vector.tensor_mul(w[:], a[:], a[:]),
                lambda w=wt, x=xt: nc.vector.tensor_scalar_add(x[:], w[:], 1.0),
                lambda x=xt, y=yt: nc.vector.tensor_mul(y[:], x[:], x[:]),
                # final square on GpSimd -- it idles otherwise, and this
                # frees ~0.7us of VectorE per D pair
                lambda y=yt, e=et: nc.gpsimd.tensor_mul(e[:], y[:], y[:]),
            ])
            dlate.append((paccs, j, et))
        if dve_work:
            dve_work.pop(0)()
        while dlate and j - dlate[0][1] >= DLAG:
            emit_av(*dlate.pop(0))
        # deferred projection pops wait a few pairs so their feature
        # subtiles have certainly landed (DMA streams are slow early on)
        if deferred and j >= 8:
            deferred.pop(0)()
    for op in dve_work:
        op()
    for args in pending + dlate:
        emit_av(*args)
    finish(paccs)


def _emit_block_finish(nc, pools, ih, out):
    """normalize both directions -> transpose to channel-major -> sum."""
    npl, aux, op = pools["np"], pools["aux"], pools["op"]
    identity = pools["identity"]
    bo_sb = pools["bo_sb"]

    def finish(paccs):
        pts = []
        for d in range(2):
            r4 = npl.tile([JCH, NSUB], F32, tag=f"r4{d}", name=f"r4{d}_{ih}")
            nc.vector.reciprocal(
                r4[:].rearrange("p (i w) -> p i w", w=1),
                paccs[d][:].rearrange("p (i w) -> p i w", w=W)[:, :, C : C + 1],
            )
            attT = npl.tile(
                [JCH, NSUB * C], F32, tag=f"attT{d}", name=f"attT{d}_{ih}"
            )
            for isub in range(NSUB):
                nc.vector.tensor_scalar_mul(
                    attT[:, C * isub : C * (isub + 1)],
                    paccs[d][:, W * isub : W * isub + C],
                    r4[:, isub : isub + 1],
                )
            pt = aux.tile([C, IH], F32, tag="aux", name=f"pt{d}_{ih}")
            for isub in range(NSUB):
                nc.tensor.transpose(
                    pt[:, JCH * isub : JCH * (isub + 1)],
                    attT[:, C * isub : C * (isub + 1)],
                    identity[:],
                )
            pts.append(pt)
        osb = op.tile([C, IH], F32, tag="osb", name=f"osb{ih}")
        nc.vector.tensor_scalar_add(osb[:], pts[0][:], bo_sb[:])
        ot = op.tile([C, IH], F32, tag="ot", name=f"ot{ih}")
        nc.vector.tensor_add(ot[:], osb[:], pts[1][:])
        nc.sync.dma_start(out[:, IH * ih : IH * (ih + 1)], ot[:])

    return finish


def _build_program(
    ctx, tc, ct, mri, qsrc_ct, qsrc_mri, wqq, wv, bo, ident, ln16, vones,
    vpat, out
):
    nc = tc.nc
    wpool = ctx.enter_context(tc.tile_pool(name="wpool", bufs=1))
    featp = ctx.enter_context(tc.tile_pool(name="feat", bufs=NFS))
    pools = {
        "qp": ctx.enter_context(tc.tile_pool(name="qp", bufs=2)),
        "vp": ctx.enter_context(tc.tile_pool(name="vp", bufs=2)),
        "ep": ctx.enter_context(tc.tile_pool(name="ep", bufs=4)),
        "epd": ctx.enter_context(tc.tile_pool(name="epd", bufs=4)),
        "dp": ctx.enter_context(tc.tile_pool(name="dp", bufs=2)),
        "np": ctx.enter_context(tc.tile_pool(name="npool", bufs=2)),
        "op": ctx.enter_context(tc.tile_pool(name="outp", bufs=2)),
        "sp": ctx.enter_context(
            tc.tile_pool(name="spsum", bufs=2, space="PSUM")
        ),
        "pap": ctx.enter_context(
            tc.tile_pool(name="paccp", bufs=1, space="PSUM")
        ),
        "aux": ctx.enter_context(
            tc.tile_pool(name="auxpsum", bufs=2, space="PSUM")
        ),
    }

    wqq_sb = wpool.tile([W, 2 * W], F16, name="wqq_sb")
    nc.sync.dma_start(wqq_sb[:], wqq[:])
    wv_sb = wpool.tile([W, 2 * W], F16, name="wv_sb")
    nc.sync.dma_start(wv_sb[:], wv[:])
    wv_hi = wpool.tile([JCH, W], F16, name="wv_hi")
    nc.sync.dma_start(wv_hi[C : 2 * C, :], wv[:C, W : 2 * W])
    bo_sb = wpool.tile([C, 1], F32, name="bo_sb")
    nc.sync.dma_start(bo_sb[:], bo[:])
    ident_sb = wpool.tile([JCH, JCH], F32, name="ident_sb")
    nc.sync.dma_start(ident_sb[:], ident[:])
    ln16_sb = wpool.tile([JCH, 1], F32, name="ln16_sb")
    nc.sync.dma_start(ln16_sb[:], ln16[:])
    vones_sb = wpool.tile([1, JCH], F16, name="vones_sb")
    nc.sync.dma_start(vones_sb[:], vones[:])
    vpat_sb = wpool.tile([1, VGW * W], F16, name="vpat_sb")
    nc.sync.dma_start(vpat_sb[:], vpat[:])
    pools["identity"] = ident_sb
    pools["bo_sb"] = bo_sb
    pools["ln16_sb"] = ln16_sb

    # tiny query-source DMAs go first so they don't queue behind the 3 MB
    # of feature DMAs (HWDGE queues are FIFO)
    qsc = pools["qp"].tile([W, NQ], F16, tag="qsrc", name="qsc")
    qsm = pools["qp"].tile([W, NQ], F16, tag="qsrc", name="qsm")
    for h in range(NIH):
        nc.sync.dma_start(
            qsc[:, IH * h : IH * (h + 1)], qsrc_ct[:, IH * h : IH * (h + 1)]
        )
        nc.sync.dma_start(
            qsm[:, IH * h : IH * (h + 1)], qsrc_mri[:, IH * h : IH * (h + 1)]
        )

    # direction 0 keys = MRI (partitions 0-63, aug copy also serves Vproj);
    # direction 1 keys = CT, replicated at partitions 64-127 for row tiling.
    # The base-0 CT aug copy only feeds Vproj-ct, so it streams through a
    # 2-subtile ring with just-in-time DMA instead of living in SBUF whole.
    fs_mri, fs_ct_hi = _emit_feat_load(nc, featp, mri, ct)

    qq_hi = pools["qp"].tile([JCH, NQ], F16, tag="qhi", name="qq_hi")
    qq_d0, qq0_units = _emit_qq_proj(nc, pools, wqq_sb, qsc, 0 * W, "qq_d0")
    qq_d1, qq1_units = _emit_qq_proj(
        nc, pools, wqq_sb, qsm, 1 * W, "qq_d1", hi_dst=qq_hi
    )
    (vT_mri, vT_ct), v_units = _emit_v_proj(
        nc, pools, wv_sb, wv_hi, fs_mri, fs_ct_hi, vones_sb, vpat_sb
    )

    # eager: all query projections + enough vT chunks to cover AV lag
    for u in qq0_units + qq1_units:
        u()
    eager_v = 3
    for g in range(eager_v):
        v_units[g]()
    deferred = list(v_units[eager_v:])

    feats = (fs_mri, fs_ct_hi)
    qqs = (qq_d0, qq_hi)
    vTs = (vT_mri, vT_ct)
    for ih in range(NIH):
        finish = _emit_block_finish(nc, pools, ih, out)
        _emit_block(nc, pools, feats, qqs, vTs, ih, deferred, finish)
    assert not deferred


def build_bass():
    nc = bacc.Bacc("TRN2", target_bir_lowering=False, debug=False)
    ct = nc.dram_tensor("ct_feat", [W, N], F16, kind="ExternalInput").ap()
    mri = nc.dram_tensor("mri_feat", [W, N], F16, kind="ExternalInput").ap()
    qsrc_ct = nc.dram_tensor("qsrc_ct", [W, NQ], F16, kind="ExternalInput").ap()
    qsrc_mri = nc.dram_tensor("qsrc_mri", [W, NQ], F16, kind="ExternalInput").ap()
    wqq = nc.dram_tensor("wqq", [W, 2 * W], F16, kind="ExternalInput").ap()
    wv = nc.dram_tensor("wv", [W, 2 * W], F16, kind="ExternalInput").ap()
    bo = nc.dram_tensor("bo", [C, 1], F32, kind="ExternalInput").ap()
    ident = nc.dram_tensor("ident", [JCH, JCH], F32, kind="ExternalInput").ap()
    ln16 = nc.dram_tensor("ln16", [JCH, 1], F32, kind="ExternalInput").ap()
    vones = nc.dram_tensor("vones", [1, JCH], F16, kind="ExternalInput").ap()
    vpat = nc.dram_tensor("vpat", [1, VGW * W], F16, kind="ExternalInput").ap()
    out = nc.dram_tensor("out", [C, NQ], F32, kind="ExternalOutput").ap()

    with tile.TileContext(nc) as tc, ExitStack() as ctx:
        _build_program(
            ctx, tc, ct, mri, qsrc_ct, qsrc_mri, wqq, wv, bo, ident, ln16,
            vones, vpat, out
        )
    nc.compile()
    return nc


def _aug(w, b):
    # (out,in) weight + (out,) bias -> lhsT-ready [w.T; b] of shape (in+1, out)
    return np.concatenate(
        [np.asarray(w, np.float32).T, np.asarray(b, np.float32)[None, :]], axis=0
    )


def _wv_pack(w, b):
    # (65, 65): [[w.T; b] | e_last]: extra column accumulates the denominator
    m = np.zeros((W, W), np.float32)
    m[:, :C] = _aug(w, b)
    m[C, C] = 1.0
    return m


def prepare_inputs(inputs):
    scale = np.float32(1.0 / np.sqrt(C))
    ct = np.asarray(inputs["ct_features"], np.float32).reshape(C, N)
    mri = np.asarray(inputs["mri_features"], np.float32).reshape(C, N)
    ones = np.ones((1, N), np.float32)
    ct_aug = np.concatenate([ct, ones], axis=0).astype(np.float16)
    mri_aug = np.concatenate([mri, ones], axis=0).astype(np.float16)
    wq_ct = _aug(np.asarray(inputs["wq_ct"]) * scale, np.asarray(inputs["bq_ct"]) * scale)
    wq_mri = _aug(np.asarray(inputs["wq_mri"]) * scale, np.asarray(inputs["bq_mri"]) * scale)
    # compose Q projection with the query-side-folded K projection (fp32 host
    # matmul, rounded to fp16 once). The K bias is dropped: it shifts every
    # score of a query row by the same amount, which softmax cancels.
    wqq = np.zeros((W, 2 * W), np.float32)
    wqq[:, 0:C] = wq_ct @ np.asarray(inputs["wk_mri"], np.float32)
    wqq[:, W : W + C] = wq_mri @ np.asarray(inputs["wk_ct"], np.float32)
    # V projections with the output projection folded in:
    # out = Wo[:, :C] @ (V_mri A0) + Wo[:, C:] @ (V_ct A1) + bo
    wo = np.asarray(inputs["wo"], np.float32)
    wv_mri, bv_mri = np.asarray(inputs["wv_mri"], np.float32), np.asarray(inputs["bv_mri"], np.float32)
    wv_ct, bv_ct = np.asarray(inputs["wv_ct"], np.float32), np.asarray(inputs["bv_ct"], np.float32)
    wv = np.concatenate(
        [_wv_pack(wo[:, :C] @ wv_mri, wo[:, :C] @ bv_mri),
         _wv_pack(wo[:, C:] @ wv_ct, wo[:, C:] @ bv_ct)],
        axis=1,
    ).astype(np.float16)
    # the V biases shift every output by a constant (softmax weights sum to
    # 1), so they fold into bo and the device projects with k=64
    bo_full = (
        np.asarray(inputs["bo"], np.float32)
        + wo[:, :C] @ bv_mri
        + wo[:, C:] @ bv_ct
    )
    bo = np.ascontiguousarray(bo_full[:, None])
    ident = np.eye(JCH, dtype=np.float32)
    vpat = np.zeros((1, VGW * W), np.float16)
    vpat[0, C::W] = 1.0

    in_maps = []
    for i in range(NCORES):
        sl = slice(NQ * i, NQ * (i + 1))
        in_maps.append(
            {
                "ct_feat": ct_aug,
                "mri_feat": mri_aug,
                "qsrc_ct": np.ascontiguousarray(ct_aug[:, sl]),
                "qsrc_mri": np.ascontiguousarray(mri_aug[:, sl]),
                "wqq": wqq.astype(np.float16),
                "wv": wv,
                "bo": bo,
                "ident": ident,
                "ln16": np.full((JCH, 1), LN16, np.float32),
                "vones": np.ones((1, JCH), np.float16),
                "vpat": vpat,
            }
        )
    return in_maps


def assemble_output(results):
    out = np.concatenate([results[i]["out"] for i in range(NCORES)], axis=1)
    return out.reshape(1, C, 8, 32, 32)


_NC_CACHE = None


def _get_nc():
    global _NC_CACHE
    if _NC_CACHE is None:
        _NC_CACHE = build_bass()
    return _NC_CACHE


def kernel(**inputs):
    nc = _get_nc()
    in_maps = prepare_inputs(inputs)
    res = run_bass_kernel_spmd(nc, in_maps, list(range(NCORES)))
    return assemble_output(res.results)


if __name__ == "__main__":
    nc = build_bass()
    print("built OK")
